# revision 4
# baseline (speedup 1.0000x reference)
"""Trainium2 Bass kernel for nn_CLRBP_23124103922240.

Math: scores[b, cls] = x[b] . W[cls] + bias[cls], softmax over 16 classes,
where W[cls] = g * tile4x4(u1 u1^T - v1 v1^T) + (1-g) * (u2 u2^T - v2 v2^T).

Key identities used:
  - tile4x4(A)[m, n] = A[m % 64, n % 64], so <X, tile(u u^T)> = uh^T X uh with
    uh = tile(u, 4); equivalently u^T (P^T X P) u with P[m, i] = (m % 64 == i).
  - v^T X v is invariant under X -> X^T, so contraction can run down X's rows.

Per sample (X = inputs[b], [256, 256], split into two 128-row chunks mc):
  stage 1 (PE): YY = [P | V2]^T X  -> YY[0:64]  = Xr  (row-pooled X, [64, 256])
                                      YY[64:96] = Y2 = V2^T X   ([32, 256])
  stage 1b (PE): Y1 = V1^T Xrp where Xrp = col-pooled Xr, done by accumulating
                 4 matmuls over 64-column slices of Xr (batched 4 samples).
  stage 2 (DVE): R1[k, b] = sum_j Y1[k, j] * (s_k V1[j, k])   (fused ttr)
                 R2[k, b] = sum_n Y2[k, n] * (s_k V2[n, k])
  stage 3 (PE): scores = R1^T G1 + R2^T G2 + 1^T b  -> [128 samples, 16]
  softmax (DVE/ACT) and DMA out.

Data-parallel over 8 NeuronCores: batch 1024 -> 128 per core.
"""

import os
import numpy as np

import concourse.bacc as bacc
import concourse.mybir as mybir
import concourse.tile as tile
from concourse.bass_utils import run_bass_kernel_spmd

N_CORES = 8
B, D, VIEW, C = 1024, 256, 4, 16
BL = B // N_CORES  # 128 samples per core
SG = 4             # samples per DMA group
NG = BL // SG      # 32 groups
F32 = mybir.dt.float32
F32R = mybir.dt.float32r

AOP = mybir.AluOpType
AFT = mybir.ActivationFunctionType
AXL = mybir.AxisListType

_cache = {}


def _build(mm_fast: bool):
    """Build + compile the SPMD program. mm_fast=True uses float32r matmuls
    (1 cyc/row at N>=256 vs 4 for fp32)."""
    if mm_fast in _cache:
        return _cache[mm_fast]

    DTX = F32R if mm_fast else F32   # dtype for matmul operands

    nc = bacc.Bacc("TRN2", target_bir_lowering=False, debug=False,
                   num_devices=N_CORES)

    x_d = nc.dram_tensor("x", [128, BL, 2, 256], DTX, kind="ExternalInput").ap()
    vp_d = nc.dram_tensor("vp", [128, 2, 128], DTX, kind="ExternalInput").ap()
    v1s_d = nc.dram_tensor("v1s", [64, 128], DTX, kind="ExternalInput").ap()
    v1pt_d = nc.dram_tensor("v1pt", [128, 64], F32, kind="ExternalInput").ap()
    v2pt_d = nc.dram_tensor("v2pt", [32, 256], F32, kind="ExternalInput").ap()
    g1_d = nc.dram_tensor("g1", [128, 16], F32, kind="ExternalInput").ap()
    g2_d = nc.dram_tensor("g2", [32, 16], F32, kind="ExternalInput").ap()
    bo_d = nc.dram_tensor("bo", [1, 144], F32, kind="ExternalInput").ap()
    out_d = nc.dram_tensor("probs", [BL, C], F32, kind="ExternalOutput").ap()

    with tile.TileContext(nc) as tc:
        with (
            tc.tile_pool(name="consts", bufs=1) as consts,
            tc.tile_pool(name="xp", bufs=6) as xpool,
            tc.tile_pool(name="xr", bufs=3) as xrpool,
            tc.tile_pool(name="scr", bufs=3) as scrpool,
            tc.tile_pool(name="fin", bufs=1) as fin,
            tc.tile_pool(name="yy", bufs=4, space="PSUM") as yypool,
            tc.tile_pool(name="y1", bufs=2, space="PSUM") as y1pool,
            tc.tile_pool(name="sc", bufs=1, space="PSUM") as scpool,
        ):
            # ---- constants ----
            # vp columns: [0:32] = V2 chunk, [32:64] = zeros, [64:128] = P.
            # Y2 then lands at PSUM partitions 0:32 (TTR requires PSUM base
            # 0) and Xr at 64:128 (ACT reads PSUM base 64; stage-1b matmul
            # runs with both operands at partition base 64).
            vp = consts.tile([128, 2, 128], DTX)
            nc.sync.dma_start(out=vp, in_=vp_d)
            v1s = consts.tile([128, 128], DTX)
            nc.sync.dma_start(out=v1s[64:128, :], in_=v1s_d)
            v1pt = consts.tile([128, 64], F32)
            nc.sync.dma_start(out=v1pt, in_=v1pt_d)
            v2pt = consts.tile([32, 256], F32)
            nc.sync.dma_start(out=v2pt, in_=v2pt_d)
            g1 = consts.tile([128, 16], F32)
            nc.sync.dma_start(out=g1, in_=g1_d)
            g2 = consts.tile([32, 16], F32)
            nc.sync.dma_start(out=g2, in_=g2_d)
            bo = consts.tile([1, 144], F32)
            nc.sync.dma_start(out=bo, in_=bo_d)

            r1 = consts.tile([128, BL], F32)   # per-rank-1-term partial scores
            r2 = consts.tile([32, BL], F32)

            for grp in range(NG):
                s0 = grp * SG
                xt = xpool.tile([128, SG, 2, 256], DTX)
                nc.sync.dma_start(out=xt, in_=x_d[:, s0:s0 + SG, :, :])

                xr4 = xrpool.tile([128, SG, 256], DTX)
                for si in range(SG):
                    s = s0 + si
                    yy = yypool.tile([128, 256], F32)
                    nc.tensor.matmul(yy, vp[:, 0, :], xt[:, si, 0, :],
                                     start=True, stop=False)
                    nc.tensor.matmul(yy, vp[:, 1, :], xt[:, si, 1, :],
                                     start=False, stop=True)
                    # Xr (row-pooled X) -> SBUF for the stage-1b matmul
                    nc.scalar.copy(xr4[64:128, si, :], yy[64:128, :])
                    # path 2 reduce: R2[k, s] = sum_n Y2[k, n] * V2pT[k, n]
                    scr2 = scrpool.tile([32, 256], F32, tag="scr2")
                    nc.vector.scalar_tensor_tensor(
                        out=scr2, in0=yy[0:32, :], scalar=1.0, in1=v2pt,
                        op0=AOP.mult, op1=AOP.mult,
                        accum_out=r2[:, s:s + 1])

                # stage 1b: Y1 = V1^T Xrp, col-pool via 4 accumulated matmuls,
                # 4 samples batched in the moving operand (N=256)
                y14 = y1pool.tile([128, SG, 64], F32)
                for q in range(4):
                    nc.tensor.matmul(y14, v1s[64:128, :],
                                     xr4[64:128, :, q * 64:(q + 1) * 64],
                                     start=(q == 0), stop=(q == 3))
                for si in range(SG):
                    s = s0 + si
                    scr1 = scrpool.tile([128, 64], F32, tag="scr1")
                    nc.vector.scalar_tensor_tensor(
                        out=scr1, in0=y14[:, si, :], scalar=1.0, in1=v1pt,
                        op0=AOP.mult, op1=AOP.mult,
                        accum_out=r1[:, s:s + 1])

            # ---- stage 3: scores [128 samples, 16] ----
            sc = scpool.tile([BL, C], F32)
            nc.tensor.matmul(sc, r1, g1, start=True, stop=False)
            nc.tensor.matmul(sc, r2, g2, start=False, stop=False)
            nc.tensor.matmul(sc, bo[:, 0:128], bo[:, 128:144],
                             start=False, stop=True)

            # ---- softmax over the 16 free elements ----
            negmax = fin.tile([BL, 1], F32)
            nc.vector.tensor_reduce(out=negmax, in_=sc, axis=AXL.X,
                                    op=AOP.max, negate=True)
            e = fin.tile([BL, C], F32)
            sume = fin.tile([BL, 1], F32)
            nc.scalar.activation(out=e, in_=sc, func=AFT.Exp, bias=negmax,
                                 scale=1.0, accum_out=sume)
            rec = fin.tile([BL, 1], F32)
            nc.vector.reciprocal(rec, sume)
            probs = fin.tile([BL, C], F32)
            nc.vector.tensor_scalar_mul(probs, e, rec)
            nc.sync.dma_start(out=out_d, in_=probs)

    nc.compile()
    _cache[mm_fast] = nc
    return nc


def _host_prep(inputs, w1, w2, l, b):
    inputs = np.asarray(inputs, dtype=np.float32)
    w1 = np.asarray(w1, dtype=np.float32)
    w2 = np.asarray(w2, dtype=np.float32)
    l = np.asarray(l, dtype=np.float32)
    b = np.asarray(b, dtype=np.float32)

    g = float(1.0 / (1.0 + np.exp(-np.float32(l[0]))))

    # path 1: rank-8 factors on the 64-block; col k = cls*8 + r
    u1, v1 = w1[:, :, 4:], w1[:, :, :4]               # [16, 64, 4]
    v1cols = np.concatenate([u1, v1], axis=2)          # [16, 64, 8]
    v1small = np.ascontiguousarray(
        v1cols.transpose(1, 0, 2).reshape(64, 128)).astype(np.float32)
    s1 = np.tile(np.array([g] * 4 + [-g] * 4, np.float32), C)        # [128]
    v1pt = np.ascontiguousarray(v1small.T * s1[:, None]).astype(np.float32)

    # path 2: rank-2 factors on full d; col k = cls*2 + {u, v}
    u2, v2 = w2[:, :, 1:2], w2[:, :, 0:1]              # [16, 256, 1]
    v2cols = np.concatenate([u2, v2], axis=2)          # [16, 256, 2]
    v2full = np.ascontiguousarray(
        v2cols.transpose(1, 0, 2).reshape(256, 32)).astype(np.float32)
    s2 = np.tile(np.array([1.0 - g, -(1.0 - g)], np.float32), C)     # [32]
    v2pt = np.ascontiguousarray(v2full.T * s2[:, None]).astype(np.float32)

    # combined stationary operand [P | V2] per row-chunk
    P = (np.arange(128)[:, None] % 64 == np.arange(64)[None, :]).astype(
        np.float32)
    vp = np.zeros((128, 2, 128), np.float32)
    for mc in range(2):
        vp[:, mc, 0:32] = v2full[mc * 128:(mc + 1) * 128, :]
        vp[:, mc, 64:128] = P

    g1 = (np.arange(128)[:, None] // 8 ==
          np.arange(C)[None, :]).astype(np.float32)
    g2 = (np.arange(32)[:, None] // 2 ==
          np.arange(C)[None, :]).astype(np.float32)
    bo = np.zeros((1, 144), np.float32)
    bo[0, 0:128] = 1.0
    bo[0, 128:144] = b

    # shard + relayout inputs: (core, p, s, mc, n)
    xs = inputs.reshape(N_CORES, BL, 2, 128, 256).transpose(0, 3, 1, 2, 4)

    shared = dict(vp=vp, v1s=v1small, v1pt=v1pt, v2pt=v2pt, g1=g1, g2=g2,
                  bo=bo)
    in_maps = []
    for core in range(N_CORES):
        m = dict(shared)
        m["x"] = np.ascontiguousarray(xs[core])
        in_maps.append(m)
    return in_maps


def kernel(inputs, w1, w2, l, b, _trace=False, _mm_fast=None):
    if _mm_fast is None:
        _mm_fast = os.environ.get("NN_MM_DTYPE", "f32r") != "f32"
    nc = _build(_mm_fast)
    in_maps = _host_prep(inputs, w1, w2, l, b)
    res = run_bass_kernel_spmd(nc, in_maps, core_ids=list(range(N_CORES)),
                               trace=_trace)
    out = np.concatenate([r["probs"] for r in res.results], axis=0)
    if _trace:
        kernel.last_results = res
    return out


# revision 7
# speedup vs baseline: 690.2229x; 690.2229x over previous
"""Trainium2 Bass kernel for nn_CLRBP_23124103922240.

Math: scores[b, cls] = x[b] . W[cls] + bias[cls], softmax over 16 classes,
where W[cls] = g * tile4x4(u1 u1^T - v1 v1^T) + (1-g) * (u2 u2^T - v2 v2^T).

Key identities used:
  - tile4x4(A)[m, n] = A[m % 64, n % 64], so <X, tile(u u^T)> = uh^T X uh with
    uh = tile(u, 4); equivalently u^T (P^T X P) u with P[m, i] = (m % 64 == i).
  - v^T X v is invariant under X -> X^T, so contraction can run down X's rows.

Per sample (X = inputs[b], [256, 256], split into two 128-row chunks mc):
  stage 1 (PE): YY = [P | V2]^T X  -> YY[0:64]  = Xr  (row-pooled X, [64, 256])
                                      YY[64:96] = Y2 = V2^T X   ([32, 256])
  stage 1b (PE): Y1 = V1^T Xrp where Xrp = col-pooled Xr, done by accumulating
                 4 matmuls over 64-column slices of Xr (batched 4 samples).
  stage 2 (DVE): R1[k, b] = sum_j Y1[k, j] * (s_k V1[j, k])   (fused ttr)
                 R2[k, b] = sum_n Y2[k, n] * (s_k V2[n, k])
  stage 3 (PE): scores = R1^T G1 + R2^T G2 + 1^T b  -> [128 samples, 16]
  softmax (DVE/ACT) and DMA out.

Data-parallel over 8 NeuronCores: batch 1024 -> 128 per core.
"""

import os
import numpy as np

import concourse.bacc as bacc
import concourse.mybir as mybir
import concourse.tile as tile
from concourse.bass_utils import run_bass_kernel_spmd

N_CORES = 8
B, D, VIEW, C = 1024, 256, 4, 16
BL = B // N_CORES  # 128 samples per core
SG = 4             # samples per DMA group
NG = BL // SG      # 32 groups
F32 = mybir.dt.float32
F32R = mybir.dt.float32r

AOP = mybir.AluOpType
AFT = mybir.ActivationFunctionType
AXL = mybir.AxisListType

_cache = {}


def _build(mm_fast: bool, reps: int = 1, sg: int = SG, xbufs: int = 6,
           yybufs: int = 4):
    """Build + compile the SPMD program. mm_fast=True uses float32r matmuls
    (1 cyc/row at N>=256 vs 4 for fp32)."""
    key = (mm_fast, reps, sg, xbufs, yybufs)
    if key in _cache:
        return _cache[key]
    ng = BL // sg

    DTX = F32R if mm_fast else F32   # dtype for matmul operands

    nc = bacc.Bacc("TRN2", target_bir_lowering=False, debug=False,
                   num_devices=N_CORES)

    x_d = nc.dram_tensor("x", [128, BL, 2, 256], DTX, kind="ExternalInput").ap()
    ca_d = nc.dram_tensor("ca", [128, 384], DTX, kind="ExternalInput").ap()
    cb_d = nc.dram_tensor("cb", [128, 496], F32, kind="ExternalInput").ap()
    out_d = nc.dram_tensor("probs", [BL, C], F32, kind="ExternalOutput").ap()

    with tile.TileContext(nc) as tc:
        with (
            tc.tile_pool(name="consts", bufs=1) as consts,
            tc.tile_pool(name="xp", bufs=xbufs) as xpool,
            tc.tile_pool(name="xr", bufs=3) as xrpool,
            tc.tile_pool(name="scr", bufs=3) as scrpool,
            tc.tile_pool(name="fin", bufs=1) as fin,
            tc.tile_pool(name="yy", bufs=yybufs, space="PSUM") as yypool,
            tc.tile_pool(name="y1", bufs=2, space="PSUM") as y1pool,
            tc.tile_pool(name="sc", bufs=1, space="PSUM") as scpool,
        ):
            # group schedule: mostly sg-sample groups, 2-sample tail
            # groups to shorten the end-of-kernel drain
            sizes = [sg] * ((BL - 4) // sg) + [2, 2]
            starts = [sum(sizes[:i]) for i in range(len(sizes))]

            # issue the first two X loads before anything else so the DMA
            # stream starts at t=0; the packed const loads overlap on other
            # queues
            pre = {}
            for grp in range(2):
                xt = xpool.tile([128, sizes[grp], 2, 256], DTX, tag="xt")
                nc.sync.dma_start(
                    out=xt, in_=x_d[:, starts[grp]:starts[grp] + sizes[grp]])
                pre[grp] = xt

            # ---- constants (2 packed DMAs) ----
            # ca: [vp (2x128) | v1s (128)]; cb: [v1pt | v2pt | g1 | g2 | bo]
            ca = consts.tile([128, 384], DTX)
            nc.sync.dma_start(out=ca, in_=ca_d)
            cb = consts.tile([128, 496], F32)
            nc.sync.dma_start(out=cb, in_=cb_d)
            vp = ca[:, 0:256].rearrange("p (m c) -> p m c", m=2)
            v1s = ca[:, 256:384]
            v1pt = cb[:, 0:64]
            v2pt = cb[0:32, 64:320]
            g1 = cb[:, 320:336]
            g2 = cb[0:32, 336:352]
            bo = cb[0:1, 352:496]

            r1 = consts.tile([128, BL], F32)   # per-rank-1-term partial scores
            r2 = consts.tile([32, BL], F32)

            for rep in range(reps):
              for grp in range(len(sizes)):
                s0 = starts[grp]
                sgi = sizes[grp]
                if rep == 0 and grp in pre:
                    xt = pre[grp]
                else:
                    xt = xpool.tile([128, sgi, 2, 256], DTX, tag="xt")
                    nc.sync.dma_start(out=xt, in_=x_d[:, s0:s0 + sgi, :, :])

                xr4 = xrpool.tile([128, sgi, 256], DTX, tag="xr4")
                for si in range(sgi):
                    s = s0 + si
                    yy = yypool.tile([128, 256], F32)
                    nc.tensor.matmul(yy, vp[:, 0, :], xt[:, si, 0, :],
                                     start=True, stop=False)
                    nc.tensor.matmul(yy, vp[:, 1, :], xt[:, si, 1, :],
                                     start=False, stop=True)
                    # Xr (row-pooled X) -> SBUF for the stage-1b matmul
                    nc.scalar.copy(xr4[64:128, si, :], yy[64:128, :])
                    # path 2 reduce: R2[k, s] = sum_n Y2[k, n] * V2pT[k, n]
                    scr2 = scrpool.tile([32, 256], F32, tag="scr2")
                    nc.vector.scalar_tensor_tensor(
                        out=scr2, in0=yy[0:32, :], scalar=1.0, in1=v2pt,
                        op0=AOP.mult, op1=AOP.mult,
                        accum_out=r2[:, s:s + 1])

                # stage 1b: Y1 = V1^T Xrp, col-pool via 4 accumulated matmuls,
                # 4 samples batched in the moving operand (N=256)
                y14 = y1pool.tile([128, sgi, 64], F32,
                                  tag="y14")
                for q in range(4):
                    nc.tensor.matmul(y14, v1s[64:128, :],
                                     xr4[64:128, :, q * 64:(q + 1) * 64],
                                     start=(q == 0), stop=(q == 3))
                for si in range(sgi):
                    s = s0 + si
                    scr1 = scrpool.tile([128, 64], F32, tag="scr1")
                    nc.vector.scalar_tensor_tensor(
                        out=scr1, in0=y14[:, si, :], scalar=1.0, in1=v1pt,
                        op0=AOP.mult, op1=AOP.mult,
                        accum_out=r1[:, s:s + 1])

            # ---- stage 3: scores [128 samples, 16] ----
            sc = scpool.tile([BL, C], F32)
            nc.tensor.matmul(sc, r1, g1, start=True, stop=False)
            nc.tensor.matmul(sc, r2, g2, start=False, stop=False)
            nc.tensor.matmul(sc, bo[:, 0:128], bo[:, 128:144],
                             start=False, stop=True)

            # ---- softmax over the 16 free elements ----
            negmax = fin.tile([BL, 1], F32)
            nc.vector.tensor_reduce(out=negmax, in_=sc, axis=AXL.X,
                                    op=AOP.max, negate=True)
            e = fin.tile([BL, C], F32)
            sume = fin.tile([BL, 1], F32)
            nc.scalar.activation(out=e, in_=sc, func=AFT.Exp, bias=negmax,
                                 scale=1.0, accum_out=sume)
            rec = fin.tile([BL, 1], F32)
            nc.vector.reciprocal(rec, sume)
            probs = fin.tile([BL, C], F32)
            nc.vector.tensor_scalar_mul(probs, e, rec)
            nc.sync.dma_start(out=out_d, in_=probs)

    nc.compile()
    _cache[key] = nc
    return nc


def _host_prep(inputs, w1, w2, l, b):
    inputs = np.asarray(inputs, dtype=np.float32)
    w1 = np.asarray(w1, dtype=np.float32)
    w2 = np.asarray(w2, dtype=np.float32)
    l = np.asarray(l, dtype=np.float32)
    b = np.asarray(b, dtype=np.float32)

    g = float(1.0 / (1.0 + np.exp(-np.float32(l[0]))))

    # path 1: rank-8 factors on the 64-block; col k = cls*8 + r
    u1, v1 = w1[:, :, 4:], w1[:, :, :4]               # [16, 64, 4]
    v1cols = np.concatenate([u1, v1], axis=2)          # [16, 64, 8]
    v1small = np.ascontiguousarray(
        v1cols.transpose(1, 0, 2).reshape(64, 128)).astype(np.float32)
    s1 = np.tile(np.array([g] * 4 + [-g] * 4, np.float32), C)        # [128]
    v1pt = np.ascontiguousarray(v1small.T * s1[:, None]).astype(np.float32)

    # path 2: rank-2 factors on full d; col k = cls*2 + {u, v}
    u2, v2 = w2[:, :, 1:2], w2[:, :, 0:1]              # [16, 256, 1]
    v2cols = np.concatenate([u2, v2], axis=2)          # [16, 256, 2]
    v2full = np.ascontiguousarray(
        v2cols.transpose(1, 0, 2).reshape(256, 32)).astype(np.float32)
    s2 = np.tile(np.array([1.0 - g, -(1.0 - g)], np.float32), C)     # [32]
    v2pt = np.ascontiguousarray(v2full.T * s2[:, None]).astype(np.float32)

    # combined stationary operand [V2 | 0 | P] per row-chunk
    P = (np.arange(128)[:, None] % 64 == np.arange(64)[None, :]).astype(
        np.float32)
    vp = np.zeros((128, 2, 128), np.float32)
    for mc in range(2):
        vp[:, mc, 0:32] = v2full[mc * 128:(mc + 1) * 128, :]
        vp[:, mc, 64:128] = P

    g1 = (np.arange(128)[:, None] // 8 ==
          np.arange(C)[None, :]).astype(np.float32)
    g2 = (np.arange(32)[:, None] // 2 ==
          np.arange(C)[None, :]).astype(np.float32)

    ca = np.zeros((128, 384), np.float32)
    ca[:, 0:256] = vp.reshape(128, 256)
    ca[64:128, 256:384] = v1small
    cb = np.zeros((128, 496), np.float32)
    cb[:, 0:64] = v1pt
    cb[0:32, 64:320] = v2pt
    cb[:, 320:336] = g1
    cb[0:32, 336:352] = g2
    cb[0, 352:480] = 1.0
    cb[0, 480:496] = b

    # shard + relayout inputs: (core, p, s, mc, n)
    xs = inputs.reshape(N_CORES, BL, 2, 128, 256).transpose(0, 3, 1, 2, 4)

    shared = dict(ca=ca, cb=cb)
    in_maps = []
    for core in range(N_CORES):
        m = dict(shared)
        m["x"] = np.ascontiguousarray(xs[core])
        in_maps.append(m)
    return in_maps


def kernel(inputs, w1, w2, l, b, _trace=False, _mm_fast=None):
    if _mm_fast is None:
        _mm_fast = os.environ.get("NN_MM_DTYPE", "f32r") != "f32"
    nc = _build(_mm_fast)
    in_maps = _host_prep(inputs, w1, w2, l, b)
    res = run_bass_kernel_spmd(nc, in_maps, core_ids=list(range(N_CORES)),
                               trace=_trace)
    out = np.concatenate([r["probs"] for r in res.results], axis=0)
    if _trace:
        kernel.last_results = res
    return out
